# revision 31
# baseline (speedup 1.0000x reference)
"""Causal single-head self-attention on 8 TRN2 NeuronCores, v4.

Sharding: 8 cores = 4 batches x 2 cores/batch, zigzag query ownership
(role 0 owns true chunks {0,3,4,7}, role 1 {1,2,5,6}; 18 causal units
each). Each core recomputes K/V for its whole batch, projects Q only
for its owned 4 chunks.

Layout/schedule (v4):
- Storage permutation: owned query chunks at storage positions 0-3
  (ascending), peer chunks at 4-7. Slot j's k-coverage is positions
  {0..j} u {4..j+4} for BOTH roles, so attention streams as a uniform
  pyramid (1,2,3,4,4,3,2,1 units/iteration) with all four flash
  accumulators resident in PSUM (4 banks).
- PV uses token-major V blocks as stationary, streams P^T 512 wide,
  accumulating O^T = [h, q] in PSUM. No PE transposes; out-projection
  consumes O^T as lhsT. V is projected token-major directly
  (x^T token tile stationary, Wv^T streaming; 56ns/MM measured), into
  a single 1-bank PSUM tile evacuated with one wide cast.
- The exp chain (QK -> ACT -> PV) leaves the PE idle ~290ns/block, and
  the PE executes its queue in order, so chunk kt+1's projection
  matmuls are MANUALLY interleaved between iteration kt's attention
  units (the Tile scheduler follows emission priority and won't do it).
- Masks: only the 16 diagonal blocks need real masks (4 distinct,
  host-built). The far position (j, j+4) is all-zero for one role and
  all-keep for the other -> folded into exp as a per-core bias
  (exp(s*scale - 1e4) == 0), zero extra ops.
- No on-chip softmax normalization: ships unnormalized out-projection
  plus per-slot bf16 exp-sum planes; host reduces the 128 k-partitions
  and divides.
- x and weights host-cast to bf16; x host-relaid to [P, chunk, dchunk,
  cols] so each 512-chunk is ONE contiguous DMA descriptor. Outputs
  split across the sync HWDGE queue and the gpsimd SWDGE queue.
"""

import numpy as np
import ml_dtypes
from contextlib import ExitStack

import concourse.bass as bass
import concourse.tile as tile
from concourse import bacc, mybir
from concourse.bass_utils import run_bass_kernel_spmd

S, B, D, H = 4096, 4, 1024, 128
P = 128
QC = 512                  # query chunk / stream width
NSLOT = 4                 # owned chunks per core
DC = D // P               # 8 d-chunks
TT = S // P               # 32 token tiles / k-blocks
NKT = S // QC             # 8 storage 512-chunks
SCALE = float(H) ** -0.5
ZBIAS = -10000.0          # exp(s*scale + ZBIAS) == 0 (masked-out role)

OWNED = {0: [0, 3, 4, 7], 1: [1, 2, 5, 6]}
SIGMA = {0: OWNED[0] + OWNED[1], 1: OWNED[1] + OWNED[0]}
# attention units (slot, storage position) processed at iteration kt.
UNITS = {kt: ([(kt, p) for p in range(kt + 1)] if kt < 4
              else [(j, kt) for j in range(kt - 4, NSLOT)])
         for kt in range(NKT)}

F32 = mybir.dt.float32
BF16 = mybir.dt.bfloat16


def _build_kernel():
    nc = bacc.Bacc("TRN2", target_bir_lowering=False, debug=False, num_devices=8)

    xb = nc.dram_tensor("xb", [P, NKT, DC, QC], BF16, kind="ExternalInput")
    wqT = nc.dram_tensor("wqT", [P, DC, H], BF16, kind="ExternalInput")
    wkT = nc.dram_tensor("wkT", [P, DC, H], BF16, kind="ExternalInput")
    wvT = nc.dram_tensor("wvT", [P, DC, H], BF16, kind="ExternalInput")
    woT = nc.dram_tensor("woT", [H, D], BF16, kind="ExternalInput")
    dmask = nc.dram_tensor("dmask", [P, 4, QC], BF16, kind="ExternalInput")
    zbias = nc.dram_tensor("zbias", [P, NSLOT], F32, kind="ExternalInput")
    out = nc.dram_tensor("out", [NSLOT * QC, D], BF16, kind="ExternalOutput")
    psums = nc.dram_tensor("psums", [P, NSLOT, QC], BF16, kind="ExternalOutput")

    with ExitStack() as ctx:
        tc = ctx.enter_context(tile.TileContext(nc))
        _body(ctx, tc, xb.ap(), wqT.ap(), wkT.ap(), wvT.ap(), woT.ap(),
              dmask.ap(), zbias.ap(), out.ap(), psums.ap())

    nc.compile()
    return nc


def _body(ctx, tc, xb, wqT, wkT, wvT, woT, dmask, zbias, out, psums):
    nc = tc.nc

    consts = ctx.enter_context(tc.tile_pool(name="consts", bufs=1))
    bigbuf = ctx.enter_context(tc.tile_pool(name="bigbuf", bufs=1))
    ptpool = ctx.enter_context(tc.tile_pool(name="pt", bufs=3))
    phpool = ctx.enter_context(tc.tile_pool(name="ph", bufs=2))
    ypool = ctx.enter_context(tc.tile_pool(name="y", bufs=4))
    psA = ctx.enter_context(tc.tile_pool(name="psA", bufs=2, space="PSUM"))
    psP = ctx.enter_context(tc.tile_pool(name="psP", bufs=2, space="PSUM"))
    psO = ctx.enter_context(tc.tile_pool(name="psO", bufs=4, space="PSUM"))

    # ---- persistent SBUF ----
    xT = bigbuf.tile([P, NKT, DC, QC], BF16)
    k_sb = bigbuf.tile([P, S], BF16)
    q_sb = bigbuf.tile([P, NSLOT * QC], BF16)
    v_sb = bigbuf.tile([P, TT, P], BF16)            # token-major V blocks
    o_sb = bigbuf.tile([P, NSLOT, QC], BF16)        # O^T [h, slot, q], unnorm
    planes = bigbuf.tile([P, NSLOT, 8, QC], BF16)   # per-unit exp partials
    wq_sb = consts.tile([P, DC, H], BF16)
    wk_sb = consts.tile([P, DC, H], BF16)
    wv_sb = consts.tile([P, DC, H], BF16)
    woT_sb = consts.tile([P, D], BF16)
    mask_sb = consts.tile([P, 4, QC], BF16)
    zb_sb = consts.tile([P, NSLOT], F32)

    # ---- startup DMAs, latency-ordered ----
    nc.gpsimd.dma_start(wk_sb[:], wkT)
    nc.gpsimd.dma_start(xT[:, 0, 0:2, :], xb[:, 0, 0:2, :])
    nc.gpsimd.dma_start(xT[:, 0, 2:8, :], xb[:, 0, 2:8, :])
    # chunks 1..7 ride the sync HWDGE queue, which carries no output
    # traffic until the first finalize (~55us): the two input streams
    # transfer in parallel and later chunks land well before their
    # projections are scheduled
    nc.sync.dma_start(xT[:, 1, :, :], xb[:, 1, :, :])
    for pair in range(2, NKT, 2):
        nc.sync.dma_start(xT[:, pair : pair + 2, :, :],
                          xb[:, pair : pair + 2, :, :])
    nc.gpsimd.dma_start(wv_sb[:], wvT)
    nc.gpsimd.dma_start(wq_sb[:], wqT)
    nc.gpsimd.dma_start(mask_sb[:], dmask)
    nc.gpsimd.dma_start(zb_sb[:], zbias)
    nc.gpsimd.dma_start(woT_sb[:], woT)

    po = {}        # slot -> open PSUM O^T accumulator
    first_pv = {}  # slot -> True until its first PV matmul

    def project_k(kt):
        ps = psP.tile([P, QC], F32, name="pp")
        for c in range(DC):
            nc.tensor.matmul(ps[:], lhsT=wk_sb[:, c, :], rhs=xT[:, kt, c, :],
                             start=(c == 0), stop=(c == DC - 1))
        nc.vector.tensor_copy(k_sb[:, bass.ts(kt, QC)], ps[:])

    def project_q(kt):
        ps = psP.tile([P, QC], F32, name="pp")
        for c in range(DC):
            nc.tensor.matmul(ps[:], lhsT=wq_sb[:, c, :], rhs=xT[:, kt, c, :],
                             start=(c == 0), stop=(c == DC - 1))
        nc.vector.tensor_copy(q_sb[:, bass.ts(kt, QC)], ps[:])

    def make_v_slices(kt):
        """token-major V for chunk kt: 32 MMs into one 1-bank PSUM tile,
        evacuated with a single wide cast. Split into 2 emission slices."""
        hold = {}

        def mms(lo, hi):
            if "psv" not in hold:
                hold["psv"] = psP.tile([P, 4, P], F32, name="pp")
            psv = hold["psv"]
            for jj in range(lo, hi):
                for c in range(DC):
                    nc.tensor.matmul(psv[:, jj, :],
                                     lhsT=xT[:, kt, c, bass.ts(jj, P)],
                                     rhs=wv_sb[:, c, :],
                                     start=(c == 0), stop=(c == DC - 1))

        def tail():
            mms(2, 4)
            nc.vector.tensor_copy(v_sb[:, bass.ds(4 * kt, 4), :], hold["psv"][:])

        return [lambda: mms(0, 2), tail]

    def proj_slices(kt):
        sl = [lambda: project_k(kt)]
        sl += make_v_slices(kt)
        if kt < NSLOT:
            sl.append(lambda: project_q(kt))
        return sl

    def attn_unit(j, p, u, fill=()):
        """slot j consumes storage chunk p (4 k-blocks); u = unit ordinal.
        `fill` lambdas are emitted one per block so independent PE work
        lands inside this unit's QK->exp->PV dependency bubbles."""
        pt_u = ptpool.tile([P, 4, QC], BF16, name="pt")
        for b in range(4):
            bk = 4 * p + b
            ps = psA.tile([P, QC], F32, name="ps")
            nc.tensor.matmul(ps[:], lhsT=k_sb[:, bass.ts(bk, P)],
                             rhs=q_sb[:, bass.ts(j, QC)], start=True, stop=True)
            bias = zb_sb[:, j : j + 1] if p == j + 4 else 0.0
            nc.scalar.activation(pt_u[:, b, :], ps[:],
                                 mybir.ActivationFunctionType.Exp,
                                 scale=SCALE, bias=bias)
            if p == j:  # diagonal: real causal mask
                nc.vector.tensor_mul(pt_u[:, b, :], pt_u[:, b, :],
                                     mask_sb[:, b, :])
            nc.tensor.matmul(po[j][:], lhsT=v_sb[:, bk, :], rhs=pt_u[:, b, :],
                             start=first_pv[j],
                             stop=(p == j + 4 and b == 3))
            first_pv[j] = False
            if b < len(fill):
                fill[b]()
        # exp-sum partial for this unit (k-partition reduction on host)
        ph = phpool.tile([P, 2, QC], BF16, name="ph")
        nc.vector.tensor_add(ph[:], pt_u[:, 0:2, :], pt_u[:, 2:4, :])
        nc.vector.tensor_add(planes[:, j, u, :], ph[:, 0, :], ph[:, 1, :])

    def finalize_pieces(j):
        """finalize_slot split into per-sub lambdas for bubble-filling."""
        return ([lambda: nc.vector.tensor_copy(o_sb[:, j, :], po[j][:])]
                + [lambda s=s: outproj_sub(j, s) for s in range(NSLOT)]
                + [lambda: ship_psums(j)])

    def outproj_sub(j, sub):
        last = j == NSLOT - 1
        tt_idx = j * NSLOT + sub
        y = ypool.tile([P, D], BF16, name="y")
        for half in range(2):
            # slot 3 runs after all attention: psA's banks are free, so
            # alternate pools for a 4-deep evacuation pipeline at the tail
            if last and half % 2:
                psy = psA.tile([P, QC], F32, name="ps")
            else:
                psy = psP.tile([P, QC], F32, name="pp")
            nc.tensor.matmul(psy[:], lhsT=o_sb[:, j, bass.ts(sub, P)],
                             rhs=woT_sb[:, bass.ts(half, QC)],
                             start=True, stop=True)
            if half == 0:  # split PSUM evacuation across DVE and ACT
                nc.vector.tensor_copy(y[:, bass.ts(half, QC)], psy[:])
            else:
                nc.scalar.copy(y[:, bass.ts(half, QC)], psy[:])
        if sub % 2 == 0 or j >= 2:  # keep the tail off the SWDGE drain
            nc.sync.dma_start(out[bass.ts(tt_idx, P), :], y[:])
        else:
            nc.gpsimd.dma_start(out[bass.ts(tt_idx, P), :], y[:])

    def ship_psums(j):
        # fold the slot's 2j+2 exp partials into plane 0, then ship it
        n = 2 * j + 2
        while n > 1:
            h = n // 2
            nc.vector.tensor_add(planes[:, j, 0:h, :], planes[:, j, 0:h, :],
                                 planes[:, j, h : 2 * h, :])
            if n % 2:
                nc.vector.tensor_add(planes[:, j, 0, :], planes[:, j, 0, :],
                                     planes[:, j, n - 1, :])
            n = h
        nc.sync.dma_start(psums[:, j, :], planes[:, j, 0, :])

    def finalize_slot(j):
        for piece in finalize_pieces(j):
            piece()

    for kt in range(NKT):
        if kt == 0:
            for s in proj_slices(0):
                s()
        nxt = proj_slices(kt + 1) if kt + 1 < NKT else []
        if kt < NSLOT:
            po[kt] = psO.tile([P, QC], F32, name="po")
            first_pv[kt] = True
        us = UNITS[kt]
        for i, (j, p) in enumerate(us):
            u = p if p <= j else j + 1 + (p - 4)
            if kt == NKT - 1:
                # slot 2's finalize was deferred here: its out-projection
                # pieces (the matmul-bearing ones) fill the last unit's
                # QK->exp->PV bubbles; the DVE-only copy goes first
                fin2 = finalize_pieces(2)
                fin2[0]()
                attn_unit(j, p, u, fill=fin2[1:5])
                fin2[5]()
            else:
                attn_unit(j, p, u)
            if i < len(nxt):
                nxt[i]()
            if p == j + 4 and j != 2:
                finalize_slot(j)
        for s in nxt[len(us):]:
            s()


_CACHED_NC = None


def _get_nc():
    global _CACHED_NC
    if _CACHED_NC is None:
        _CACHED_NC = _build_kernel()
    return _CACHED_NC


def _make_core_inputs(x, wqT, wkT, wvT, woT, core):
    # tolerate f32 weights from older harnesses
    wqT, wkT, wvT, woT = (np.asarray(w).astype(ml_dtypes.bfloat16)
                          for w in (wqT, wkT, wvT, woT))
    b, role = core // 2, core % 2
    sigma = SIGMA[role]
    perm = np.concatenate([np.arange(QC) + c * QC for c in sigma])
    xp = np.asarray(x[perm, b, :], np.float32)           # [S, D] storage order
    xb = np.ascontiguousarray(
        xp.reshape(NKT, QC, DC, P).transpose(3, 0, 2, 1)
    ).astype(ml_dtypes.bfloat16)                          # [P, NKT, DC, QC]

    # diagonal masks: block b keeps (1.0) where q >= b*128 + k
    kk = np.arange(P)[:, None]
    qq = np.arange(QC)[None, :]
    dmask = np.zeros((P, 4, QC), ml_dtypes.bfloat16)
    for bb in range(4):
        dmask[:, bb, :] = (qq >= bb * P + kk)
    # far-position (j, j+4) bias: peer chunk kept iff its true index < o_j
    zb = np.zeros((P, NSLOT), np.float32)
    for j in range(NSLOT):
        if OWNED[1 - role][j] > OWNED[role][j]:
            zb[:, j] = ZBIAS
    return {"xb": xb, "wqT": wqT, "wkT": wkT, "wvT": wvT, "woT": woT,
            "dmask": dmask, "zbias": zb}


def _w_pch(w):
    """(H, D) weight -> [p, c, h] bf16 layout for a contiguous SBUF load."""
    return np.ascontiguousarray(
        np.asarray(w, np.float32).T.reshape(DC, P, H).transpose(1, 0, 2)
    ).astype(ml_dtypes.bfloat16)


def kernel(x, Wq, Wk, Wv, Wo):
    x = np.asarray(x, dtype=np.float32)
    wqT = _w_pch(Wq)
    wkT = _w_pch(Wk)
    wvT = _w_pch(Wv)
    woT = np.ascontiguousarray(np.asarray(Wo, np.float32).T).astype(
        ml_dtypes.bfloat16)

    nc = _get_nc()
    in_maps = [_make_core_inputs(x, wqT, wkT, wvT, woT, i) for i in range(8)]
    res = run_bass_kernel_spmd(nc, in_maps, list(range(8))).results

    out = np.empty((S, B, D), np.float32)
    for core in range(8):
        b, role = core // 2, core % 2
        co = np.asarray(res[core]["out"]).astype(np.float32)
        ps = np.asarray(res[core]["psums"]).astype(np.float32)  # [P, NSLOT, QC]
        for j in range(NSLOT):
            denom = ps[:, j, :].sum(axis=0)                     # [QC]
            c_j = OWNED[role][j]
            out[c_j * QC : (c_j + 1) * QC, b, :] = (
                co[j * QC : (j + 1) * QC, :] / denom[:, None]
            )
    return out


# revision 32
# speedup vs baseline: 1.1396x; 1.1396x over previous
"""Causal single-head self-attention on 8 TRN2 NeuronCores, v4.

Sharding: 8 cores = 4 batches x 2 cores/batch, zigzag query ownership
(role 0 owns true chunks {0,3,4,7}, role 1 {1,2,5,6}; 18 causal units
each). Each core recomputes K/V for its whole batch, projects Q only
for its owned 4 chunks.

Layout/schedule (v4):
- Storage permutation: owned query chunks at storage positions 0-3
  (ascending), peer chunks at 4-7. Slot j's k-coverage is positions
  {0..j} u {4..j+4} for BOTH roles, so attention streams as a uniform
  pyramid (1,2,3,4,4,3,2,1 units/iteration) with all four flash
  accumulators resident in PSUM (4 banks).
- PV uses token-major V blocks as stationary, streams P^T 512 wide,
  accumulating O^T = [h, q] in PSUM. No PE transposes; out-projection
  consumes O^T as lhsT. V is projected token-major directly
  (x^T token tile stationary, Wv^T streaming; 56ns/MM measured), into
  a single 1-bank PSUM tile evacuated with one wide cast.
- The exp chain (QK -> ACT -> PV) leaves the PE idle ~290ns/block, and
  the PE executes its queue in order, so chunk kt+1's projection
  matmuls are MANUALLY interleaved between iteration kt's attention
  units (the Tile scheduler follows emission priority and won't do it).
- Masks: only the 16 diagonal blocks need real masks (4 distinct,
  host-built). The far position (j, j+4) is all-zero for one role and
  all-keep for the other -> folded into exp as a per-core bias
  (exp(s*scale - 1e4) == 0), zero extra ops.
- No on-chip softmax normalization: ships unnormalized out-projection
  plus per-slot bf16 exp-sum planes; host reduces the 128 k-partitions
  and divides.
- x and weights host-cast to bf16; x host-relaid to [P, chunk, dchunk,
  cols] so each 512-chunk is ONE contiguous DMA descriptor. Outputs
  split across the sync HWDGE queue and the gpsimd SWDGE queue.
"""

import numpy as np
import ml_dtypes
from contextlib import ExitStack

import concourse.bass as bass
import concourse.tile as tile
from concourse import bacc, mybir
from concourse.bass_utils import run_bass_kernel_spmd

S, B, D, H = 4096, 4, 1024, 128
P = 128
QC = 512                  # query chunk / stream width
NSLOT = 4                 # owned chunks per core
DC = D // P               # 8 d-chunks
TT = S // P               # 32 token tiles / k-blocks
NKT = S // QC             # 8 storage 512-chunks
SCALE = float(H) ** -0.5
ZBIAS = -10000.0          # exp(s*scale + ZBIAS) == 0 (masked-out role)

OWNED = {0: [0, 3, 4, 7], 1: [1, 2, 5, 6]}
SIGMA = {0: OWNED[0] + OWNED[1], 1: OWNED[1] + OWNED[0]}
# attention units (slot, storage position) processed at iteration kt.
UNITS = {kt: ([(kt, p) for p in range(kt + 1)] if kt < 4
              else [(j, kt) for j in range(kt - 4, NSLOT)])
         for kt in range(NKT)}

F32 = mybir.dt.float32
BF16 = mybir.dt.bfloat16


def _build_kernel():
    nc = bacc.Bacc("TRN2", target_bir_lowering=False, debug=False, num_devices=8)

    xb = nc.dram_tensor("xb", [P, NKT, DC, QC], BF16, kind="ExternalInput")
    wqT = nc.dram_tensor("wqT", [P, DC, H], BF16, kind="ExternalInput")
    wkT = nc.dram_tensor("wkT", [P, DC, H], BF16, kind="ExternalInput")
    wvT = nc.dram_tensor("wvT", [P, DC, H], BF16, kind="ExternalInput")
    woT = nc.dram_tensor("woT", [H, D], BF16, kind="ExternalInput")
    dmask = nc.dram_tensor("dmask", [P, 4, QC], BF16, kind="ExternalInput")
    zbias = nc.dram_tensor("zbias", [P, NSLOT], F32, kind="ExternalInput")
    out = nc.dram_tensor("out", [NSLOT * QC, D], BF16, kind="ExternalOutput")
    psums = nc.dram_tensor("psums", [P, NSLOT, QC], BF16, kind="ExternalOutput")

    with ExitStack() as ctx:
        tc = ctx.enter_context(tile.TileContext(nc))
        _body(ctx, tc, xb.ap(), wqT.ap(), wkT.ap(), wvT.ap(), woT.ap(),
              dmask.ap(), zbias.ap(), out.ap(), psums.ap())

    nc.compile()
    return nc


def _body(ctx, tc, xb, wqT, wkT, wvT, woT, dmask, zbias, out, psums):
    nc = tc.nc

    consts = ctx.enter_context(tc.tile_pool(name="consts", bufs=1))
    bigbuf = ctx.enter_context(tc.tile_pool(name="bigbuf", bufs=1))
    ptpool = ctx.enter_context(tc.tile_pool(name="pt", bufs=3))
    phpool = ctx.enter_context(tc.tile_pool(name="ph", bufs=2))
    ypool = ctx.enter_context(tc.tile_pool(name="y", bufs=4))
    psA = ctx.enter_context(tc.tile_pool(name="psA", bufs=2, space="PSUM"))
    psP = ctx.enter_context(tc.tile_pool(name="psP", bufs=2, space="PSUM"))
    psO = ctx.enter_context(tc.tile_pool(name="psO", bufs=4, space="PSUM"))

    # ---- persistent SBUF ----
    xT = bigbuf.tile([P, NKT, DC, QC], BF16)
    k_sb = bigbuf.tile([P, S], BF16)
    q_sb = bigbuf.tile([P, NSLOT * QC], BF16)
    v_sb = bigbuf.tile([P, TT, P], BF16)            # token-major V blocks
    o_sb = bigbuf.tile([P, NSLOT, QC], BF16)        # O^T [h, slot, q], unnorm
    planes = bigbuf.tile([P, NSLOT, 8, QC], BF16)   # per-unit exp partials
    wq_sb = consts.tile([P, DC, H], BF16)
    wk_sb = consts.tile([P, DC, H], BF16)
    wv_sb = consts.tile([P, DC, H], BF16)
    woT_sb = consts.tile([P, D], BF16)
    mask_sb = consts.tile([P, 4, QC], BF16)
    zb_sb = consts.tile([P, NSLOT], F32)

    # ---- startup DMAs, latency-ordered ----
    nc.gpsimd.dma_start(wk_sb[:], wkT)
    nc.gpsimd.dma_start(xT[:, 0, 0:2, :], xb[:, 0, 0:2, :])
    nc.gpsimd.dma_start(xT[:, 0, 2:8, :], xb[:, 0, 2:8, :])
    nc.gpsimd.dma_start(wv_sb[:], wvT)
    nc.gpsimd.dma_start(wq_sb[:], wqT)
    nc.gpsimd.dma_start(zb_sb[:], zbias)
    nc.gpsimd.dma_start(mask_sb[:], dmask)
    nc.gpsimd.dma_start(xT[:, 1, :, :], xb[:, 1, :, :])
    nc.gpsimd.dma_start(woT_sb[:], woT)
    for pair in range(2, NKT, 2):
        nc.gpsimd.dma_start(xT[:, pair : pair + 2, :, :],
                            xb[:, pair : pair + 2, :, :])

    po = {}        # slot -> open PSUM O^T accumulator
    first_pv = {}  # slot -> True until its first PV matmul

    def project_k(kt):
        ps = psP.tile([P, QC], F32, name="pp")
        for c in range(DC):
            nc.tensor.matmul(ps[:], lhsT=wk_sb[:, c, :], rhs=xT[:, kt, c, :],
                             start=(c == 0), stop=(c == DC - 1))
        nc.vector.tensor_copy(k_sb[:, bass.ts(kt, QC)], ps[:])

    def project_q(kt):
        ps = psP.tile([P, QC], F32, name="pp")
        for c in range(DC):
            nc.tensor.matmul(ps[:], lhsT=wq_sb[:, c, :], rhs=xT[:, kt, c, :],
                             start=(c == 0), stop=(c == DC - 1))
        nc.vector.tensor_copy(q_sb[:, bass.ts(kt, QC)], ps[:])

    def make_v_slices(kt):
        """token-major V for chunk kt: 32 MMs into one 1-bank PSUM tile,
        evacuated with a single wide cast. Split into 2 emission slices."""
        hold = {}

        def mms(lo, hi):
            if "psv" not in hold:
                hold["psv"] = psP.tile([P, 4, P], F32, name="pp")
            psv = hold["psv"]
            for jj in range(lo, hi):
                for c in range(DC):
                    nc.tensor.matmul(psv[:, jj, :],
                                     lhsT=xT[:, kt, c, bass.ts(jj, P)],
                                     rhs=wv_sb[:, c, :],
                                     start=(c == 0), stop=(c == DC - 1))

        def tail():
            mms(2, 4)
            nc.vector.tensor_copy(v_sb[:, bass.ds(4 * kt, 4), :], hold["psv"][:])

        return [lambda: mms(0, 2), tail]

    def proj_slices(kt):
        sl = [lambda: project_k(kt)]
        sl += make_v_slices(kt)
        if kt < NSLOT:
            sl.append(lambda: project_q(kt))
        return sl

    def attn_unit(j, p, u, fill=()):
        """slot j consumes storage chunk p (4 k-blocks); u = unit ordinal.
        `fill` lambdas are emitted one per block so independent PE work
        lands inside this unit's QK->exp->PV dependency bubbles."""
        pt_u = ptpool.tile([P, 4, QC], BF16, name="pt")
        for b in range(4):
            bk = 4 * p + b
            ps = psA.tile([P, QC], F32, name="ps")
            nc.tensor.matmul(ps[:], lhsT=k_sb[:, bass.ts(bk, P)],
                             rhs=q_sb[:, bass.ts(j, QC)], start=True, stop=True)
            bias = zb_sb[:, j : j + 1] if p == j + 4 else 0.0
            nc.scalar.activation(pt_u[:, b, :], ps[:],
                                 mybir.ActivationFunctionType.Exp,
                                 scale=SCALE, bias=bias)
            if p == j:  # diagonal: real causal mask
                nc.vector.tensor_mul(pt_u[:, b, :], pt_u[:, b, :],
                                     mask_sb[:, b, :])
            nc.tensor.matmul(po[j][:], lhsT=v_sb[:, bk, :], rhs=pt_u[:, b, :],
                             start=first_pv[j],
                             stop=(p == j + 4 and b == 3))
            first_pv[j] = False
            if b < len(fill):
                fill[b]()
        # exp-sum partial for this unit (k-partition reduction on host)
        ph = phpool.tile([P, 2, QC], BF16, name="ph")
        nc.vector.tensor_add(ph[:], pt_u[:, 0:2, :], pt_u[:, 2:4, :])
        nc.vector.tensor_add(planes[:, j, u, :], ph[:, 0, :], ph[:, 1, :])

    def finalize_pieces(j):
        """finalize_slot split into per-sub lambdas for bubble-filling."""
        return ([lambda: nc.vector.tensor_copy(o_sb[:, j, :], po[j][:])]
                + [lambda s=s: outproj_sub(j, s) for s in range(NSLOT)]
                + [lambda: ship_psums(j)])

    def outproj_sub(j, sub):
        last = j == NSLOT - 1
        tt_idx = j * NSLOT + sub
        y = ypool.tile([P, D], BF16, name="y")
        for half in range(2):
            # slot 3 runs after all attention: psA's banks are free, so
            # alternate pools for a 4-deep evacuation pipeline at the tail
            if last and half % 2:
                psy = psA.tile([P, QC], F32, name="ps")
            else:
                psy = psP.tile([P, QC], F32, name="pp")
            nc.tensor.matmul(psy[:], lhsT=o_sb[:, j, bass.ts(sub, P)],
                             rhs=woT_sb[:, bass.ts(half, QC)],
                             start=True, stop=True)
            if half == 0:  # split PSUM evacuation across DVE and ACT
                nc.vector.tensor_copy(y[:, bass.ts(half, QC)], psy[:])
            else:
                nc.scalar.copy(y[:, bass.ts(half, QC)], psy[:])
        if sub % 2 == 0 or j >= 2:  # keep the tail off the SWDGE drain
            nc.sync.dma_start(out[bass.ts(tt_idx, P), :], y[:])
        else:
            nc.gpsimd.dma_start(out[bass.ts(tt_idx, P), :], y[:])

    def ship_psums(j):
        # fold the slot's 2j+2 exp partials into plane 0, then ship it
        n = 2 * j + 2
        while n > 1:
            h = n // 2
            nc.vector.tensor_add(planes[:, j, 0:h, :], planes[:, j, 0:h, :],
                                 planes[:, j, h : 2 * h, :])
            if n % 2:
                nc.vector.tensor_add(planes[:, j, 0, :], planes[:, j, 0, :],
                                     planes[:, j, n - 1, :])
            n = h
        nc.sync.dma_start(psums[:, j, :], planes[:, j, 0, :])

    def finalize_slot(j):
        for piece in finalize_pieces(j):
            piece()

    for kt in range(NKT):
        if kt == 0:
            for s in proj_slices(0):
                s()
        nxt = proj_slices(kt + 1) if kt + 1 < NKT else []
        if kt < NSLOT:
            po[kt] = psO.tile([P, QC], F32, name="po")
            first_pv[kt] = True
        us = UNITS[kt]
        for i, (j, p) in enumerate(us):
            u = p if p <= j else j + 1 + (p - 4)
            if kt == NKT - 1:
                # slot 2's finalize was deferred here: its out-projection
                # pieces (the matmul-bearing ones) fill the last unit's
                # QK->exp->PV bubbles; the DVE-only copy goes first
                fin2 = finalize_pieces(2)
                fin2[0]()
                attn_unit(j, p, u, fill=fin2[1:5])
                fin2[5]()
            else:
                attn_unit(j, p, u)
            if i < len(nxt):
                nxt[i]()
            if p == j + 4 and j != 2:
                finalize_slot(j)
        for s in nxt[len(us):]:
            s()


_CACHED_NC = None


def _get_nc():
    global _CACHED_NC
    if _CACHED_NC is None:
        _CACHED_NC = _build_kernel()
    return _CACHED_NC


def _make_core_inputs(x, wqT, wkT, wvT, woT, core):
    # tolerate f32 weights from older harnesses
    wqT, wkT, wvT, woT = (np.asarray(w).astype(ml_dtypes.bfloat16)
                          for w in (wqT, wkT, wvT, woT))
    b, role = core // 2, core % 2
    sigma = SIGMA[role]
    perm = np.concatenate([np.arange(QC) + c * QC for c in sigma])
    xp = np.asarray(x[perm, b, :], np.float32)           # [S, D] storage order
    xb = np.ascontiguousarray(
        xp.reshape(NKT, QC, DC, P).transpose(3, 0, 2, 1)
    ).astype(ml_dtypes.bfloat16)                          # [P, NKT, DC, QC]

    # diagonal masks: block b keeps (1.0) where q >= b*128 + k
    kk = np.arange(P)[:, None]
    qq = np.arange(QC)[None, :]
    dmask = np.zeros((P, 4, QC), ml_dtypes.bfloat16)
    for bb in range(4):
        dmask[:, bb, :] = (qq >= bb * P + kk)
    # far-position (j, j+4) bias: peer chunk kept iff its true index < o_j
    zb = np.zeros((P, NSLOT), np.float32)
    for j in range(NSLOT):
        if OWNED[1 - role][j] > OWNED[role][j]:
            zb[:, j] = ZBIAS
    return {"xb": xb, "wqT": wqT, "wkT": wkT, "wvT": wvT, "woT": woT,
            "dmask": dmask, "zbias": zb}


def _w_pch(w):
    """(H, D) weight -> [p, c, h] bf16 layout for a contiguous SBUF load."""
    return np.ascontiguousarray(
        np.asarray(w, np.float32).T.reshape(DC, P, H).transpose(1, 0, 2)
    ).astype(ml_dtypes.bfloat16)


def kernel(x, Wq, Wk, Wv, Wo):
    x = np.asarray(x, dtype=np.float32)
    wqT = _w_pch(Wq)
    wkT = _w_pch(Wk)
    wvT = _w_pch(Wv)
    woT = np.ascontiguousarray(np.asarray(Wo, np.float32).T).astype(
        ml_dtypes.bfloat16)

    nc = _get_nc()
    in_maps = [_make_core_inputs(x, wqT, wkT, wvT, woT, i) for i in range(8)]
    res = run_bass_kernel_spmd(nc, in_maps, list(range(8))).results

    out = np.empty((S, B, D), np.float32)
    for core in range(8):
        b, role = core // 2, core % 2
        co = np.asarray(res[core]["out"]).astype(np.float32)
        ps = np.asarray(res[core]["psums"]).astype(np.float32)  # [P, NSLOT, QC]
        for j in range(NSLOT):
            denom = ps[:, j, :].sum(axis=0)                     # [QC]
            c_j = OWNED[role][j]
            out[c_j * QC : (c_j + 1) * QC, b, :] = (
                co[j * QC : (j + 1) * QC, :] / denom[:, None]
            )
    return out
